# revision 1
# baseline (speedup 1.0000x reference)
"""Trainium2 Bass kernel for a 2-layer heterogeneous GNN (HGNN).

Graph: users/items (50000 each), 3 relations (follows: u->u, rates: u->i,
ratedby: i->u), 800000 edges per relation, GraphConv with norm='both',
HeteroGraphConv mean aggregation over relations per destination type.

Strategy (8 NeuronCores, SPMD single program):
  - Destination-node sharding: core c owns destination rows
    [c*6250, (c+1)*6250) of both the user and item tables, for every
    relation. Edges are partitioned by destination on the host and sorted by
    (dst window of 128 rows, src chunk); padding makes the per-window edge
    counts uniform so one static program serves all 8 cores.
  - feat = (x @ W) * rsqrt(out_deg) is computed sharded (each core does its
    6250 source rows) and AllGather'ed into a replicated bf16 table.
  - The SpMM (segment_sum of gathered rows) runs as: dma_gather of edge
    source rows (HBM -> SBUF, edges land on partitions), a one-hot matrix S
    built per 128-edge tile with a DVE is_equal against an iota row, and a
    TensorE matmul S^T @ G accumulated in PSUM per 128-destination-row
    window.  Window flush applies rsqrt(in_deg), bias, mean-over-relations,
    and ReLU (layer 1 only).
  - Layer-2 features are produced window-by-window from the layer-1 flush
    (PE transpose to get h^T for the stationary operand), AllGather'ed, and
    aggregated the same way.  Outputs stay fp32.

All numeric heavy lifting (matmuls, gathers, segment sums, normalization)
runs on device; the host only computes graph metadata (degrees, edge
partitioning/sorting, index/layout packing) and casts/transposes inputs.
"""

import math

import numpy as np
import ml_dtypes

import concourse.bacc as bacc
import concourse.bass as bass
import concourse.tile as tile
from concourse import mybir
from concourse.bass_utils import run_bass_kernel_spmd

BF16 = mybir.dt.bfloat16
F32 = mybir.dt.float32
I16 = mybir.dt.int16

NCORE = 8
N = 50000
E = 800000
D_IN = 256
D_HID = 256
D_OUT = 128
SLAB = N // NCORE          # 6250 destination rows per core
WPC = (SLAB + 127) // 128  # 49 windows of 128 dst rows
SLAB_PAD = WPC * 128       # 6272
PADN = NCORE * SLAB_PAD    # 50176 rows in gathered (padded) tables
CHUNK0 = 32768             # int16 index limit: src chunk boundary

RELS = ("follows", "rates", "ratedby")
SRC_IS_USER = {"follows": True, "rates": True, "ratedby": False}

WG1 = 2  # dst windows per gather call group, layer 1 (elem 512B)
WG2 = 4  # layer 2 (elem 256B)

_CACHE = {}
LAST_RESULT = None
DBG_PHASES = "ABC"   # debug: which phases to emit
DBG_NWIN = None      # debug: limit number of dst windows in phases B/C
DBG_NO_MM = False    # debug: skip S-build + matmul + flush (gathers only)
DBG_FLUSH = 4        # debug: 0 none, 1 t1, 2 +relu, 3 +transpose, 4 full


# ----------------------------------------------------------------- host prep


def _pack_idx(lin, ncols16):
    """[8, L] int16 -> [8, 128, L//16] wrapped (elem i at [i%16, i//16]),
    replicated across the 8 Q7-core partition groups."""
    a = lin.reshape(NCORE, ncols16, 16).transpose(0, 2, 1)  # [8, 16, cols]
    return np.ascontiguousarray(np.tile(a, (1, 8, 1)))


def _pack_dst(lin, ntiles):
    """[8, L] -> [8, 128, L//128] bf16 (elem j at [j%128, j//128])."""
    return np.ascontiguousarray(
        lin.reshape(NCORE, ntiles, 128).transpose(0, 2, 1)
    ).astype(ml_dtypes.bfloat16)


def _prep_relation(src, dst):
    """Partition/sort one relation's edges for the 8 cores.

    Returns dict with TA/TB (per-window A/B slot budgets, multiples of 128)
    and packed idx/dstrel streams per core.
    """
    src = np.asarray(src).astype(np.int64)
    dst = np.asarray(dst).astype(np.int64)

    core = dst // SLAB
    dst_loc = dst - core * SLAB
    w = dst_loc >> 7
    dstrel = (dst_loc & 127).astype(np.float32)
    src_pad = src + (SLAB_PAD - SLAB) * (src // SLAB)  # row in padded table
    chunk = (src_pad >= CHUNK0).astype(np.int64)

    key = ((core * WPC + w) * 2 + chunk).astype(np.int64)
    order = np.argsort(key, kind="stable")
    key_s = key[order]
    counts = np.bincount(key_s, minlength=NCORE * WPC * 2)
    starts = np.concatenate(([0], np.cumsum(counts)[:-1]))
    pos = np.arange(E, dtype=np.int64) - starts[key_s]

    cnt = counts.reshape(NCORE, WPC, 2)
    maxA = int(cnt[:, :, 0].max())
    maxB = int(cnt[:, :, 1].max())
    TA = ((maxA + 127) // 128) * 128
    TB = ((maxB + 127) // 128) * 128

    core_s = core[order]
    w_s = w[order]
    chunk_s = chunk[order]
    sp_s = src_pad[order]
    dr_s = dstrel[order]

    idxA = np.full((NCORE, WPC * TA), -1, np.int16)
    dstA = np.full((NCORE, WPC * TA), -1.0, np.float32)
    idxB = np.full((NCORE, WPC * TB), -1, np.int16)
    dstB = np.full((NCORE, WPC * TB), -1.0, np.float32)

    mA = chunk_s == 0
    linA = w_s[mA] * TA + pos[mA]
    idxA[core_s[mA], linA] = sp_s[mA].astype(np.int16)
    dstA[core_s[mA], linA] = dr_s[mA]
    mB = ~mA
    linB = w_s[mB] * TB + pos[mB]
    idxB[core_s[mB], linB] = (sp_s[mB] - CHUNK0).astype(np.int16)
    dstB[core_s[mB], linB] = dr_s[mB]

    cntA = cnt[:, :, 0].copy()  # [NCORE, WPC]
    cntB = cnt[:, :, 1].copy()
    # a 1024-chunk with zero valid slots would emit an empty gather; give
    # such chunks one valid (masked) slot pointing at row 0.
    for c in range(NCORE):
        for w in range(WPC):
            for k0 in range(0, TA, 1024):
                if cntA[c, w] <= k0:
                    idxA[c, w * TA + k0] = 0
            for k0 in range(0, TB, 1024):
                if cntB[c, w] <= k0:
                    idxB[c, w * TB + k0] = 0
    return {
        "TA": TA,
        "TB": TB,
        "cntA": cntA,
        "cntB": cntB,
        "idxA": _pack_idx(idxA, WPC * TA // 16),
        "idxB": _pack_idx(idxB, WPC * TB // 16),
        "dstA": _pack_dst(dstA, WPC * TA // 128),
        "dstB": _pack_dst(dstB, WPC * TB // 128),
    }


def _rs_slabs(v):
    """rsqrt vector [N] fp32 -> per-core [8, 128, 49] slab layout."""
    vp = np.ones(PADN, np.float32)
    vp_view = vp.reshape(NCORE, SLAB_PAD)
    vp_view[:, :SLAB] = v.reshape(NCORE, SLAB)
    return np.ascontiguousarray(
        vp_view.reshape(NCORE, WPC, 128).transpose(0, 2, 1)
    )


def _xt_slabs(x, d):
    """x [N, d] fp32 -> per-core transposed bf16 slabs [8, 128, d//128, SLAB_PAD]."""
    out = np.zeros((NCORE, 128, d // 128, SLAB_PAD), ml_dtypes.bfloat16)
    xs = x.reshape(NCORE, SLAB, d)
    for c in range(NCORE):
        xt = xs[c].T.astype(ml_dtypes.bfloat16)  # [d, SLAB]
        out[c, :, :, :SLAB] = xt.reshape(d // 128, 128, SLAB).transpose(1, 0, 2)
    return out


def _call_counts(meta):
    """Per-core int32 call-count streams matching seg_pass emission order.
    Also returns per-call 'full on every core' flags (register not needed)."""
    rows = [[] for _ in range(NCORE)]
    for wg in (WG1, WG2):
        for wlist in _groups(wg):
            for r in RELS:
                ta, tb = meta[r]["TA"], meta[r]["TB"]
                for c in range(NCORE):
                    row = rows[c]
                    for w in wlist:
                        cA = int(meta[r]["cntA"][c, w])
                        k0 = 0
                        while k0 < ta:
                            k1 = min(k0 + 1024, ta)
                            row.append(max(1, min(cA - k0, k1 - k0)))
                            k0 = k1
                    if tb:
                        for w in wlist:
                            cB = int(meta[r]["cntB"][c, w])
                            k0 = 0
                            while k0 < tb:
                                k1 = min(k0 + 1024, tb)
                                row.append(max(1, min(cB - k0, k1 - k0)))
                                k0 = k1
    n = len(rows[0])
    arr = np.zeros((NCORE, 1, n), np.int32)
    for c in range(NCORE):
        arr[c, 0, :] = rows[c]
    return arr


def _groups(wg):
    nw = WPC if DBG_NWIN is None else min(DBG_NWIN, WPC)
    gs = []
    w0 = 0
    while w0 < nw:
        gs.append(list(range(w0, min(w0 + wg, nw))))
        w0 += wg
    return gs


# ------------------------------------------------------------ device program


import contextlib


@contextlib.contextmanager
def _nullcm():
    yield


def _build(meta):
    nc = bacc.Bacc("TRN2", debug=False, dynamic_dma_scratch_size=32768, num_swdge_queues=4)

    inp = {}

    def din(name, shape, dt):
        inp[name] = nc.dram_tensor(name, list(shape), dt, kind="ExternalInput")
        return inp[name]

    din("xtu", (128, 2, SLAB_PAD), BF16)
    din("xti", (128, 2, SLAB_PAD), BF16)
    for r in RELS:
        din(f"w1_{r}", (128, 2, D_HID), BF16)
        din(f"w2_{r}", (128, 2, D_OUT), BF16)
        din(f"rso_{r}", (128, WPC), F32)
        ta, tb = meta[r]["TA"], meta[r]["TB"]
        din(f"ixa_{r}", (128, WPC * ta // 16), I16)
        din(f"ixb_{r}", (128, WPC * tb // 16), I16)
        din(f"dsa_{r}", (128, WPC * ta // 128), BF16)
        din(f"dsb_{r}", (128, WPC * tb // 128), BF16)
    din("rsif", (128, WPC), F32)   # 0.5 * rs_in follows   (user rows)
    din("rsirb", (128, WPC), F32)  # 0.5 * rs_in ratedby   (user rows)
    din("rsii", (128, WPC), F32)   # rs_in rates           (item rows)
    din("iota", (128, 128), BF16)
    din("ident", (128, 128), BF16)
    din("b1u", (128, D_HID), F32)
    din("b1i", (128, D_HID), F32)
    din("b2u", (128, D_OUT), F32)
    din("b2i", (128, D_OUT), F32)
    ncalls = meta["ncalls"]
    din("ccnt", (1, ncalls), mybir.dt.int32)

    ou = nc.dram_tensor("ou", [SLAB_PAD, D_OUT], F32, kind="ExternalOutput")
    oi = nc.dram_tensor("oi", [SLAB_PAD, D_OUT], F32, kind="ExternalOutput")

    f1 = {r: nc.dram_tensor(f"f1_{r}", [SLAB_PAD, D_HID], BF16) for r in RELS}
    f1f = {
        r: nc.dram_tensor(f"f1f_{r}", [PADN, D_HID], BF16, addr_space="Shared")
        for r in RELS
    }
    f2 = {r: nc.dram_tensor(f"f2_{r}", [SLAB_PAD, D_OUT], BF16) for r in RELS}
    f2f = {
        r: nc.dram_tensor(f"f2f_{r}", [PADN, D_OUT], BF16, addr_space="Shared")
        for r in RELS
    }

    eq = mybir.AluOpType.is_equal
    mult = mybir.AluOpType.mult
    add = mybir.AluOpType.add
    rg = [list(range(NCORE))]

    with tile.TileContext(nc) as tc:
        with tc.tile_pool(name="const", bufs=1) as cpool:
            w1_sb = {}
            w2_sb = {}
            rso_sb = {}
            dsa_sb = {}
            dsb_sb = {}
            for r in RELS:
                w1_sb[r] = cpool.tile([128, 2, D_HID], BF16, tag=f"w1{r}", name=f"w1sb_{r}")
                nc.sync.dma_start(w1_sb[r][:], inp[f"w1_{r}"][:])
                w2_sb[r] = cpool.tile([128, 2, D_OUT], BF16, tag=f"w2{r}", name=f"w2sb_{r}")
                nc.sync.dma_start(w2_sb[r][:], inp[f"w2_{r}"][:])
                rso_sb[r] = cpool.tile([128, WPC], F32, tag=f"rso{r}", name=f"rsosb_{r}")
                nc.sync.dma_start(rso_sb[r][:], inp[f"rso_{r}"][:])
                ta, tb = meta[r]["TA"], meta[r]["TB"]
                dsa_sb[r] = cpool.tile([128, WPC * ta // 128], BF16, tag=f"da{r}", name=f"dsasb_{r}")
                nc.sync.dma_start(dsa_sb[r][:], inp[f"dsa_{r}"][:])
                dsb_sb[r] = cpool.tile([128, WPC * tb // 128], BF16, tag=f"db{r}", name=f"dsbsb_{r}")
                nc.sync.dma_start(dsb_sb[r][:], inp[f"dsb_{r}"][:])
            rsif = cpool.tile([128, WPC], F32, tag="rsif")
            nc.sync.dma_start(rsif[:], inp["rsif"][:])
            rsirb = cpool.tile([128, WPC], F32, tag="rsirb")
            nc.sync.dma_start(rsirb[:], inp["rsirb"][:])
            rsii = cpool.tile([128, WPC], F32, tag="rsii")
            nc.sync.dma_start(rsii[:], inp["rsii"][:])
            iota_sb = cpool.tile([128, 128], BF16, tag="iota")
            nc.sync.dma_start(iota_sb[:], inp["iota"][:])
            ident_sb = cpool.tile([128, 128], BF16, tag="ident")
            nc.sync.dma_start(ident_sb[:], inp["ident"][:])
            b1u = cpool.tile([128, D_HID], F32, tag="b1u")
            nc.sync.dma_start(b1u[:], inp["b1u"][:])
            b1i = cpool.tile([128, D_HID], F32, tag="b1i")
            nc.sync.dma_start(b1i[:], inp["b1i"][:])
            b2u = cpool.tile([128, D_OUT], F32, tag="b2u")
            nc.sync.dma_start(b2u[:], inp["b2u"][:])
            b2i = cpool.tile([128, D_OUT], F32, tag="b2i")
            nc.sync.dma_start(b2i[:], inp["b2i"][:])
            ccnt = cpool.tile([1, ncalls], mybir.dt.int32, tag="ccnt")
            nc.sync.dma_start(ccnt[:], inp["ccnt"][:])

            # ---------------- phase A: layer-1 features (sharded) + AllGather
            with (
                tc.tile_pool(name="xt", bufs=1) as xpool,
                tc.tile_pool(name="psA", bufs=4, space="PSUM") as psA,
                tc.tile_pool(name="fA", bufs=4) as fA,
            ):
                xtu = xpool.tile([128, 2, SLAB_PAD], BF16, tag="xtu")
                nc.sync.dma_start(xtu[:], inp["xtu"][:])
                xti = xpool.tile([128, 2, SLAB_PAD], BF16, tag="xti")
                nc.sync.dma_start(xti[:], inp["xti"][:])
                for r in RELS:
                    xs = xtu if SRC_IS_USER[r] else xti
                    for nt in range(WPC):
                        ps = psA.tile([128, D_HID], F32, tag="psA")
                        for kc in range(2):
                            nc.tensor.matmul(
                                ps[:],
                                xs[:, kc, nt * 128 : (nt + 1) * 128],
                                w1_sb[r][:, kc, :],
                                start=(kc == 0),
                                stop=(kc == 1),
                            )
                        ft = fA.tile([128, D_HID], BF16, tag="fA")
                        nc.vector.tensor_scalar_mul(
                            ft[:], ps[:], rso_sb[r][:, nt : nt + 1]
                        )
                        nc.sync.dma_start(
                            f1[r][nt * 128 : (nt + 1) * 128, :], ft[:]
                        )
                for r in RELS:
                    nc.gpsimd.collective_compute(
                        "AllGather",
                        mybir.AluOpType.bypass,
                        replica_groups=rg,
                        ins=[f1[r].ap().opt()],
                        outs=[f1f[r].ap().opt()],
                    )

            # ---------------- phase B: layer-1 SpMM + layer-2 features
            qctr = [0]
            _fresh = {}

            def _first_zero(t, tag, bufs):
                n = _fresh.get(tag, 0)
                if n < bufs:
                    nc.vector.memset(t[:], 0.0)
                    _fresh[tag] = n + 1

            def _build_s(spool, tag, dst_ap, nt):
                """One DVE op: S[e, t, m] = (iota[m] == dstrel[e, t])."""
                st = spool.tile([128, nt, 128], BF16, tag=tag, name=tag)
                nc.vector.tensor_tensor(
                    st[:],
                    iota_sb[:, :]
                    .rearrange("p (o f) -> p o f", o=1)
                    .broadcast_to([128, nt, 128]),
                    dst_ap.rearrange("p (t o) -> p t o", o=1)
                    .broadcast_to([128, nt, 128]),
                    mybir.AluOpType.is_equal,
                )
                return st

            callfull = meta["callfull"]

            def _split_gather(gt, src_ap, idx_t, n, d, seg):
                """Single-packet (<=1024 idx) gather sub-calls, split per
                window segment of `seg` slots; valid counts via register
                (skipped when the call is statically full on all cores)."""
                for s0 in range(0, n, seg):
                    k0 = s0
                    while k0 < s0 + seg:
                        k1 = min(k0 + 1024, s0 + seg)
                        if callfull[qctr[0]]:
                            creg = k1 - k0
                        else:
                            creg = nc.gpsimd.value_load(
                                ccnt[0:1, qctr[0] : qctr[0] + 1]
                            )
                        nc.gpsimd.dma_gather(
                            gt[:, k0 // 128 : k1 // 128, :],
                            src_ap,
                            idx_t[:, k0 // 16 : k1 // 16],
                            k1 - k0,
                            creg,
                            d,
                            single_packet=True,
                            queue_num=qctr[0] % 4,
                        )
                        qctr[0] += 1
                        k0 = k1

            def seg_pass(layer, gsrc, d, wg, flush):
                """One gather+segment pass over all windows.

                gsrc[r]: full DRAM table AP for relation r; d: feature dim;
                flush(w, psums) consumes the 3 accumulated PSUM tiles."""
                for wlist in _groups(wg):
                    nw = len(wlist)
                    w0 = wlist[0]
                    gt = {}
                    for r in RELS:
                        ta, tb = meta[r]["TA"], meta[r]["TB"]
                        nta, ntb = ta // 128, tb // 128
                        ixa = gpool.tile(
                            [128, wg * ta // 16], I16, tag=f"ixa{layer}"
                        )
                        nc.sync.dma_start(
                            ixa[:, : nw * ta // 16],
                            inp[f"ixa_{r}"][
                                :, w0 * ta // 16 : (w0 + nw) * ta // 16
                            ],
                        )
                        ga = gpool.tile(
                            [128, wg * nta, d], BF16, tag=f"ga{layer}"
                        )
                        _first_zero(ga, f"ga{layer}", 5)
                        _split_gather(
                            ga, gsrc[r][0:CHUNK0, :], ixa, nw * ta, d, ta
                        )
                        gb = None
                        if ntb:
                            ixb = gpool.tile(
                                [128, wg * tb // 16], I16, tag=f"ixb{layer}"
                            )
                            nc.sync.dma_start(
                                ixb[:, : nw * tb // 16],
                                inp[f"ixb_{r}"][
                                    :, w0 * tb // 16 : (w0 + nw) * tb // 16
                                ],
                            )
                            gb = gpool.tile(
                                [128, wg * ntb, d], BF16, tag=f"gb{layer}"
                            )
                            _first_zero(gb, f"gb{layer}", 5)
                            _split_gather(
                                gb, gsrc[r][CHUNK0:PADN, :], ixb, nw * tb, d, tb
                            )
                        gt[r] = (ga, gb, nta, ntb)
                    if DBG_NO_MM:
                        continue
                    for j, w in enumerate(wlist):
                        psums = {}
                        for r in RELS:
                            ga, gb, nta, ntb = gt[r]
                            ps = pspool.tile([128, d], F32, tag=f"ps{layer}{r}", name=f"ps{layer}{r}")
                            psums[r] = ps
                            sa = _build_s(
                                spool, f"SA{layer}",
                                dsa_sb[r][:, w * nta : (w + 1) * nta], nta,
                            )
                            for t in range(nta):
                                nc.tensor.matmul(
                                    ps[:],
                                    sa[:, t, :],
                                    ga[:, j * nta + t, :],
                                    start=(t == 0),
                                    stop=(ntb == 0 and t == nta - 1),
                                )
                            if ntb:
                                sb_ = _build_s(
                                    spool, f"SB{layer}",
                                    dsb_sb[r][:, w * ntb : (w + 1) * ntb], ntb,
                                )
                                for t in range(ntb):
                                    nc.tensor.matmul(
                                        ps[:],
                                        sb_[:, t, :],
                                        gb[:, j * ntb + t, :],
                                        start=False,
                                        stop=(t == ntb - 1),
                                    )
                        flush(w, psums)

            if DBG_PHASES == "A":
                phase_b = phase_c = False
            elif DBG_PHASES == "AB":
                phase_b, phase_c = True, False
            else:
                phase_b = phase_c = True
            with (
                tc.tile_pool(name="g1", bufs=4) as gpool,
                tc.tile_pool(name="ps1", bufs=1, space="PSUM") as pspool,
                tc.tile_pool(name="ps2p", bufs=2, space="PSUM") as ps2pool,
                tc.tile_pool(name="s1", bufs=4) as spool,
                tc.tile_pool(name="fl1", bufs=3) as flpool,
                tc.tile_pool(name="pst", bufs=2, space="PSUM") as pstpool,
            ):

                def flush1(w, psums):
                    if DBG_FLUSH == 0:
                        return
                    t1 = flpool.tile([128, D_HID], F32, tag="t1")
                    nc.vector.scalar_tensor_tensor(
                        t1[:], psums["follows"][:], rsif[:, w : w + 1], b1u[:],
                        mult, add,
                    )
                    if DBG_FLUSH == 1:
                        return
                    t2 = flpool.tile([128, D_HID], F32, tag="t2")
                    nc.vector.scalar_tensor_tensor(
                        t2[:], psums["ratedby"][:], rsirb[:, w : w + 1], t1[:],
                        mult, add,
                    )
                    hu = flpool.tile([128, D_HID], BF16, tag="hu")
                    nc.vector.tensor_scalar_max(hu[:], t2[:], 0.0)
                    t3 = flpool.tile([128, D_HID], F32, tag="t3")
                    nc.vector.scalar_tensor_tensor(
                        t3[:], psums["rates"][:], rsii[:, w : w + 1], b1i[:],
                        mult, add,
                    )
                    hi = flpool.tile([128, D_HID], BF16, tag="hi")
                    nc.vector.tensor_scalar_max(hi[:], t3[:], 0.0)
                    if DBG_FLUSH == 2:
                        return
                    # transpose h tiles (PE) for the layer-2 feature matmuls
                    hts = {}
                    for nm, h in (("u", hu), ("i", hi)):
                        ht = flpool.tile([128, 2, 128], BF16, tag=f"ht{nm}", name=f"ht{nm}")
                        for half in range(2):
                            pst = pstpool.tile([128, 128], BF16, tag="pst")
                            nc.tensor.transpose(
                                pst[:],
                                h[:, half * 128 : (half + 1) * 128],
                                ident_sb[:],
                            )
                            nc.vector.tensor_copy(ht[:, half, :], pst[:])
                        hts[nm] = ht
                    if DBG_FLUSH == 3:
                        return
                    for r in RELS:
                        ht = hts["u"] if SRC_IS_USER[r] else hts["i"]
                        ps2 = ps2pool.tile([128, D_OUT], F32, tag="ps2")
                        for kc in range(2):
                            nc.tensor.matmul(
                                ps2[:],
                                ht[:, kc, :],
                                w2_sb[r][:, kc, :],
                                start=(kc == 0),
                                stop=(kc == 1),
                            )
                        f2t = flpool.tile([128, D_OUT], BF16, tag="f2t")
                        nc.vector.tensor_scalar_mul(
                            f2t[:], ps2[:], rso_sb[r][:, w : w + 1]
                        )
                        nc.sync.dma_start(
                            f2[r][w * 128 : (w + 1) * 128, :], f2t[:]
                        )

                if phase_b:
                    seg_pass(1, {r: f1f[r] for r in RELS}, D_HID, WG1, flush1)
                if phase_c:
                    for r in RELS:
                        nc.gpsimd.collective_compute(
                            "AllGather",
                            mybir.AluOpType.bypass,
                            replica_groups=rg,
                            ins=[f2[r].ap().opt()],
                            outs=[f2f[r].ap().opt()],
                        )

            # ---------------- phase C: layer-2 SpMM -> outputs
            with (
                tc.tile_pool(name="g2", bufs=5) as gpool,
                tc.tile_pool(name="ps2c", bufs=2, space="PSUM") as pspool,
                tc.tile_pool(name="s2", bufs=4) as spool,
                tc.tile_pool(name="fl2", bufs=3) as flpool,
            ):

                def flush2(w, psums):
                    t1 = flpool.tile([128, D_OUT], F32, tag="o1")
                    nc.vector.scalar_tensor_tensor(
                        t1[:], psums["follows"][:], rsif[:, w : w + 1], b2u[:],
                        mult, add,
                    )
                    out_u = flpool.tile([128, D_OUT], F32, tag="ou")
                    nc.vector.scalar_tensor_tensor(
                        out_u[:], psums["ratedby"][:], rsirb[:, w : w + 1],
                        t1[:], mult, add,
                    )
                    nc.sync.dma_start(ou[w * 128 : (w + 1) * 128, :], out_u[:])
                    out_i = flpool.tile([128, D_OUT], F32, tag="oiT")
                    nc.vector.scalar_tensor_tensor(
                        out_i[:], psums["rates"][:], rsii[:, w : w + 1], b2i[:],
                        mult, add,
                    )
                    nc.sync.dma_start(oi[w * 128 : (w + 1) * 128, :], out_i[:])

                if phase_c:
                    seg_pass(2, {r: f2f[r] for r in RELS}, D_OUT, WG2, flush2)

    nc.compile()
    return nc


# ------------------------------------------------------------------- kernel


def prepare(inputs):
    """Host-side prep: returns (meta, in_maps)."""
    meta = {}
    for r in RELS:
        meta[r] = _prep_relation(inputs[f"src_{r}"], inputs[f"dst_{r}"])
    ccnt = _call_counts(meta)
    meta["ncalls"] = ccnt.shape[2]
    # a call is register-free iff every core's valid count equals the
    # chunk length; chunk lengths are identical across cores by design,
    # so compare against the max (which equals the static chunk length
    # only if some core is full -- conservative: require all equal AND
    # reconstruct the chunk length the same way _split_gather does.
    lens = []
    for wg2 in (WG1, WG2):
        for wlist2 in _groups(wg2):
            for r2 in RELS:
                ta2, tb2 = meta[r2]["TA"], meta[r2]["TB"]
                for _w in wlist2:
                    k0 = 0
                    while k0 < ta2:
                        k1 = min(k0 + 1024, ta2)
                        lens.append(k1 - k0)
                        k0 = k1
                if tb2:
                    for _w in wlist2:
                        k0 = 0
                        while k0 < tb2:
                            k1 = min(k0 + 1024, tb2)
                            lens.append(k1 - k0)
                            k0 = k1
    assert len(lens) == ccnt.shape[2], (len(lens), ccnt.shape[2])
    meta["callfull"] = [
        bool((ccnt[:, 0, i] == lens[i]).all()) for i in range(len(lens))
    ]

    bf = ml_dtypes.bfloat16
    x_user = np.asarray(inputs["x_user"], np.float32)
    x_item = np.asarray(inputs["x_item"], np.float32)

    xtu = _xt_slabs(x_user, D_IN)
    xti = _xt_slabs(x_item, D_IN)

    rs_out = {}
    rs_in = {}
    for r in RELS:
        src = np.asarray(inputs[f"src_{r}"]).astype(np.int64)
        dst = np.asarray(inputs[f"dst_{r}"]).astype(np.int64)
        rs_out[r] = _rs_slabs(
            1.0
            / np.sqrt(np.maximum(np.bincount(src, minlength=N), 1.0)).astype(
                np.float32
            )
        )
        rs_in[r] = 1.0 / np.sqrt(
            np.maximum(np.bincount(dst, minlength=N), 1.0)
        ).astype(np.float32)

    rsif = _rs_slabs(0.5 * rs_in["follows"])
    rsirb = _rs_slabs(0.5 * rs_in["ratedby"])
    rsii = _rs_slabs(rs_in["rates"])

    iota = np.broadcast_to(np.arange(128, dtype=np.float32), (128, 128)).astype(bf)
    ident = np.eye(128, dtype=np.float32).astype(bf)

    b1u = np.broadcast_to(
        0.5
        * (
            np.asarray(inputs["b1_follows"], np.float32)
            + np.asarray(inputs["b1_ratedby"], np.float32)
        ),
        (128, D_HID),
    ).astype(np.float32)
    b1i = np.broadcast_to(
        np.asarray(inputs["b1_rates"], np.float32), (128, D_HID)
    ).astype(np.float32)
    b2u = np.broadcast_to(
        0.5
        * (
            np.asarray(inputs["b2_follows"], np.float32)
            + np.asarray(inputs["b2_ratedby"], np.float32)
        ),
        (128, D_OUT),
    ).astype(np.float32)
    b2i = np.broadcast_to(
        np.asarray(inputs["b2_rates"], np.float32), (128, D_OUT)
    ).astype(np.float32)

    w1 = {
        r: np.ascontiguousarray(
            np.asarray(inputs[f"W1_{r}"], np.float32)
            .astype(bf)
            .reshape(2, 128, D_HID)
            .transpose(1, 0, 2)
        )
        for r in RELS
    }
    w2 = {
        r: np.ascontiguousarray(
            np.asarray(inputs[f"W2_{r}"], np.float32)
            .astype(bf)
            .reshape(2, 128, D_OUT)
            .transpose(1, 0, 2)
        )
        for r in RELS
    }

    in_maps = []
    for c in range(NCORE):
        m = {
            "xtu": np.ascontiguousarray(xtu[c]),
            "xti": np.ascontiguousarray(xti[c]),
            "rsif": rsif[c],
            "rsirb": rsirb[c],
            "rsii": rsii[c],
            "iota": iota,
            "ident": ident,
            "b1u": b1u,
            "b1i": b1i,
            "b2u": b2u,
            "b2i": b2i,
            "ccnt": ccnt[c],
        }
        for r in RELS:
            m[f"w1_{r}"] = w1[r]
            m[f"w2_{r}"] = w2[r]
            m[f"rso_{r}"] = rs_out[r][c]
            m[f"ixa_{r}"] = meta[r]["idxA"][c]
            m[f"ixb_{r}"] = meta[r]["idxB"][c]
            m[f"dsa_{r}"] = meta[r]["dstA"][c]
            m[f"dsb_{r}"] = meta[r]["dstB"][c]
        in_maps.append(m)
    return meta, in_maps


def kernel(**inputs):
    key = tuple(
        (int(np.asarray(inputs[f"src_{r}"][:97]).sum()),
         int(np.asarray(inputs[f"dst_{r}"][:97]).sum()))
        for r in RELS
    )
    meta, in_maps = prepare(inputs)
    if key not in _CACHE:
        _CACHE[key] = _build(meta)
    nc = _CACHE[key]

    global LAST_RESULT
    res = run_bass_kernel_spmd(nc, in_maps, list(range(NCORE)))
    LAST_RESULT = res

    o_u = np.concatenate(
        [res.results[c]["ou"][:SLAB] for c in range(NCORE)], axis=0
    )
    o_i = np.concatenate(
        [res.results[c]["oi"][:SLAB] for c in range(NCORE)], axis=0
    )
    return (o_u, o_i)



# revision 10
# speedup vs baseline: 1.1326x; 1.1326x over previous
"""Trainium2 Bass kernel for a 2-layer heterogeneous GNN (HGNN).

Graph: users/items (50000 each), 3 relations (follows: u->u, rates: u->i,
ratedby: i->u), 800000 edges per relation, GraphConv with norm='both',
HeteroGraphConv mean aggregation over relations per destination type.

Strategy (8 NeuronCores, SPMD single program):
  - Destination-node sharding: core c owns destination rows
    [c*6250, (c+1)*6250) of both the user and item tables, for every
    relation. Edges are partitioned by destination on the host and sorted by
    (dst window of 128 rows, src half-slab); per-(window, half) slot budgets
    are shared across cores (max over cores, rounded to 128) so one static
    program serves all 8 cores.
  - feat = (x @ W) * rsqrt(out_deg) is computed sharded (each core does its
    6250 source rows) and AllGather'ed into replicated bf16 tables.  Each
    relation-layer table is split in two half-slab tables (8*3072 and 8*3200
    rows, both < 32768 so int16 gather indices work directly) so each half
    can AllGather as soon as it is produced and overlap downstream work.
  - The SpMM (segment_sum of gathered rows) runs as: one big dma_gather per
    (relation, half, window-group) of edge source rows (HBM -> SBUF, edges
    land on partitions; padded slots gather row 0 and are masked), a one-hot
    matrix S built per 128-edge tile with a DVE is_equal against an iota row,
    and TensorE matmuls S^T @ G accumulated in PSUM per 128-destination-row
    window.  Window flush applies rsqrt(in_deg), bias, mean-over-relations
    (DVE) and ReLU / psum-scaling copies (Scalar engine; layer 1 only).
  - Layer-2 features are produced window-by-window from the layer-1 flush
    (PE transpose to get h^T for the stationary operand), AllGather'ed per
    half mid-pass, and aggregated the same way.  Outputs stay fp32.

All numeric heavy lifting (matmuls, gathers, segment sums, normalization)
runs on device; the host only computes graph metadata (degrees, edge
partitioning/sorting, index/layout packing) and casts/transposes inputs.
"""

import numpy as np
import ml_dtypes

import concourse.bacc as bacc
import concourse.bass as bass
import concourse.tile as tile
from concourse import mybir
from concourse.bass_utils import run_bass_kernel_spmd

BF16 = mybir.dt.bfloat16
F32 = mybir.dt.float32
I16 = mybir.dt.int16
AF = mybir.ActivationFunctionType

NCORE = 8
N = 50000
E = 800000
D_IN = 256
D_HID = 256
D_OUT = 128
SLAB = N // NCORE          # 6250 destination rows per core
WPC = (SLAB + 127) // 128  # 49 windows of 128 dst rows
SLAB_PAD = WPC * 128       # 6272
H0W = 24                   # windows in half 0
H1W = WPC - H0W            # 25 windows in half 1
H0 = H0W * 128             # 3072 rows per half-0 slab
H1 = H1W * 128             # 3200 rows per half-1 slab
T0 = NCORE * H0            # 24576 rows in half-0 table (< 32768: int16 ok)
T1 = NCORE * H1            # 25600 rows in half-1 table

RELS = ("follows", "rates", "ratedby")
SRC_IS_USER = {"follows": True, "rates": True, "ratedby": False}

WG1 = 2  # dst windows per gather call group, layer 1 (elem 512B)
WG2 = 4  # layer 2 (elem 256B)
MAXCALL = 1024  # max indices per dma_gather sub-call

_CACHE = {}
LAST_RESULT = None


# ----------------------------------------------------------------- host prep


def _pack_idx(lin, ncols16):
    """[8, L] int16 -> [8, 128, L//16] wrapped (elem i at [i%16, i//16]),
    replicated across the 8 Q7-core partition groups."""
    a = lin.reshape(NCORE, ncols16, 16).transpose(0, 2, 1)  # [8, 16, cols]
    return np.ascontiguousarray(np.tile(a, (1, 8, 1)))


def _pack_dst(lin, ntiles):
    """[8, L] -> [8, 128, L//128] bf16 (elem j at [j%128, j//128])."""
    return np.ascontiguousarray(
        lin.reshape(NCORE, ntiles, 128).transpose(0, 2, 1)
    ).astype(ml_dtypes.bfloat16)


def _prep_relation(src, dst):
    """Partition/sort one relation's edges for the 8 cores.

    Per-(window, half) tile budgets shared across cores; padded idx slots
    point at row 0 (gathered garbage is masked by the one-hot S whose padded
    dstrel is -1)."""
    src = np.asarray(src).astype(np.int64)
    dst = np.asarray(dst).astype(np.int64)

    core = dst // SLAB
    dst_loc = dst - core * SLAB
    w = dst_loc >> 7
    dstrel = (dst_loc & 127).astype(np.float32)
    sc = src // SLAB
    j = src - sc * SLAB                      # 0..6249 within source slab
    half = (j >= H0).astype(np.int64)
    row = np.where(half == 0, sc * H0 + j, sc * H1 + (j - H0))

    key = ((core * WPC + w) * 2 + half).astype(np.int64)
    order = np.argsort(key, kind="stable")
    key_s = key[order]
    counts = np.bincount(key_s, minlength=NCORE * WPC * 2)
    starts = np.concatenate(([0], np.cumsum(counts)[:-1]))
    pos = np.arange(E, dtype=np.int64) - starts[key_s]

    cnt = counts.reshape(NCORE, WPC, 2)
    # tiles per (window, half): shared across cores
    nt = [
        np.maximum(1, -(-cnt[:, :, h].max(axis=0) // 128)).astype(np.int64)
        for h in (0, 1)
    ]  # each [WPC]
    off = [np.concatenate(([0], np.cumsum(nt[h] * 128)[:-1])) for h in (0, 1)]
    tot = [int((nt[h] * 128).sum()) for h in (0, 1)]

    core_s = core[order]
    w_s = w[order]
    half_s = half[order]
    row_s = row[order]
    dr_s = dstrel[order]

    idx = [np.zeros((NCORE, tot[h]), np.int16) for h in (0, 1)]
    dstv = [np.full((NCORE, tot[h]), -1.0, np.float32) for h in (0, 1)]
    for h in (0, 1):
        m = half_s == h
        lin = off[h][w_s[m]] + pos[m]
        idx[h][core_s[m], lin] = row_s[m].astype(np.int16)
        dstv[h][core_s[m], lin] = dr_s[m]

    return {
        "nt": nt,
        "off": off,
        "tot": tot,
        "ntmax": [int(nt[h].max()) for h in (0, 1)],
        "idx": [_pack_idx(idx[h], tot[h] // 16) for h in (0, 1)],
        "dst": [_pack_dst(dstv[h], tot[h] // 128) for h in (0, 1)],
    }


def _rs_slabs(v):
    """rsqrt vector [N] fp32 -> per-core [8, 128, 49] slab layout."""
    vp = np.ones(NCORE * SLAB_PAD, np.float32)
    vp_view = vp.reshape(NCORE, SLAB_PAD)
    vp_view[:, :SLAB] = v.reshape(NCORE, SLAB)
    return np.ascontiguousarray(
        vp_view.reshape(NCORE, WPC, 128).transpose(0, 2, 1)
    )


def _xt_slabs(x, d):
    """x [N, d] fp32 -> per-core transposed bf16 slabs [8, 128, d//128, SLAB_PAD]."""
    out = np.zeros((NCORE, 128, d // 128, SLAB_PAD), ml_dtypes.bfloat16)
    xs = x.reshape(NCORE, SLAB, d)
    for c in range(NCORE):
        xt = xs[c].T.astype(ml_dtypes.bfloat16)  # [d, SLAB]
        out[c, :, :, :SLAB] = xt.reshape(d // 128, 128, SLAB).transpose(1, 0, 2)
    return out


def _groups(wg):
    gs = []
    w0 = 0
    while w0 < WPC:
        gs.append(list(range(w0, min(w0 + wg, WPC))))
        w0 += wg
    return gs


# ------------------------------------------------------------ device program


def _build(meta):
    nc = bacc.Bacc(
        "TRN2", debug=False, dynamic_dma_scratch_size=32768, num_swdge_queues=4
    )

    inp = {}

    def din(name, shape, dt):
        inp[name] = nc.dram_tensor(name, list(shape), dt, kind="ExternalInput")
        return inp[name]

    din("xtu", (128, 2, SLAB_PAD), BF16)
    din("xti", (128, 2, SLAB_PAD), BF16)
    for r in RELS:
        din(f"w1_{r}", (128, 2, D_HID), BF16)
        din(f"w2_{r}", (128, 2, D_OUT), BF16)
        din(f"rso_{r}", (128, WPC), F32)
        for h in (0, 1):
            tot = meta[r]["tot"][h]
            din(f"ix{h}_{r}", (128, tot // 16), I16)
            din(f"ds{h}_{r}", (128, tot // 128), BF16)
    din("rsif", (128, WPC), F32)   # 0.5 * rs_in follows   (user rows)
    din("rsirb", (128, WPC), F32)  # 0.5 * rs_in ratedby   (user rows)
    din("rsii", (128, WPC), F32)   # rs_in rates           (item rows)
    din("iota", (128, 128), BF16)
    din("ident", (128, 128), BF16)
    din("b1u", (128, D_HID), F32)
    din("b1i", (128, D_HID), F32)
    din("b2u", (128, D_OUT), F32)
    din("b2i", (128, D_OUT), F32)

    ou = nc.dram_tensor("ou", [SLAB_PAD, D_OUT], F32, kind="ExternalOutput")
    oi = nc.dram_tensor("oi", [SLAB_PAD, D_OUT], F32, kind="ExternalOutput")

    HROWS = (H0, H1)
    TROWS = (T0, T1)
    f1 = {r: [nc.dram_tensor(f"f1{h}_{r}", [HROWS[h], D_HID], BF16) for h in (0, 1)]
          for r in RELS}
    f1f = {r: [nc.dram_tensor(f"f1f{h}_{r}", [TROWS[h], D_HID], BF16,
                              addr_space="Shared") for h in (0, 1)]
           for r in RELS}
    f2 = {r: [nc.dram_tensor(f"f2{h}_{r}", [HROWS[h], D_OUT], BF16) for h in (0, 1)]
          for r in RELS}
    f2f = {r: [nc.dram_tensor(f"f2f{h}_{r}", [TROWS[h], D_OUT], BF16,
                              addr_space="Shared") for h in (0, 1)]
           for r in RELS}

    mult = mybir.AluOpType.mult
    add = mybir.AluOpType.add
    rg = [list(range(NCORE))]
    qctr = [0]

    def ag(src_t, dst_t):
        nc.gpsimd.collective_compute(
            "AllGather",
            mybir.AluOpType.bypass,
            replica_groups=rg,
            ins=[src_t.ap().opt()],
            outs=[dst_t.ap().opt()],
        )

    with tile.TileContext(nc) as tc:
        with tc.tile_pool(name="const", bufs=1) as cpool:
            w1_sb = {}
            w2_sb = {}
            rso_sb = {}
            ds_sb = {}
            for r in RELS:
                w1_sb[r] = cpool.tile([128, 2, D_HID], BF16, tag=f"w1{r}", name=f"w1sb_{r}")
                nc.sync.dma_start(w1_sb[r][:], inp[f"w1_{r}"][:])
                w2_sb[r] = cpool.tile([128, 2, D_OUT], BF16, tag=f"w2{r}", name=f"w2sb_{r}")
                nc.sync.dma_start(w2_sb[r][:], inp[f"w2_{r}"][:])
                rso_sb[r] = cpool.tile([128, WPC], F32, tag=f"rso{r}", name=f"rsosb_{r}")
                nc.sync.dma_start(rso_sb[r][:], inp[f"rso_{r}"][:])
                ds_sb[r] = []
                for h in (0, 1):
                    t = cpool.tile([128, meta[r]["tot"][h] // 128], BF16,
                                   tag=f"ds{h}{r}", name=f"dssb{h}_{r}")
                    nc.sync.dma_start(t[:], inp[f"ds{h}_{r}"][:])
                    ds_sb[r].append(t)
            rsif = cpool.tile([128, WPC], F32, tag="rsif")
            nc.sync.dma_start(rsif[:], inp["rsif"][:])
            rsirb = cpool.tile([128, WPC], F32, tag="rsirb")
            nc.sync.dma_start(rsirb[:], inp["rsirb"][:])
            rsii = cpool.tile([128, WPC], F32, tag="rsii")
            nc.sync.dma_start(rsii[:], inp["rsii"][:])
            iota_sb = cpool.tile([128, 128], BF16, tag="iota")
            nc.sync.dma_start(iota_sb[:], inp["iota"][:])
            ident_sb = cpool.tile([128, 128], BF16, tag="ident")
            nc.sync.dma_start(ident_sb[:], inp["ident"][:])
            b1u = cpool.tile([128, D_HID], F32, tag="b1u")
            nc.sync.dma_start(b1u[:], inp["b1u"][:])
            b1i = cpool.tile([128, D_HID], F32, tag="b1i")
            nc.sync.dma_start(b1i[:], inp["b1i"][:])
            b2u = cpool.tile([128, D_OUT], F32, tag="b2u")
            nc.sync.dma_start(b2u[:], inp["b2u"][:])
            b2i = cpool.tile([128, D_OUT], F32, tag="b2i")
            nc.sync.dma_start(b2i[:], inp["b2i"][:])

            # ---------------- phase A: layer-1 features (sharded) + AllGather
            # emitted half-by-half so each half-table AllGather fires as soon
            # as its windows are written (phase-B gathers consume A halves of
            # all relations first).
            with (
                tc.tile_pool(name="xt", bufs=1) as xpool,
                tc.tile_pool(name="psA", bufs=4, space="PSUM") as psA,
                tc.tile_pool(name="fA", bufs=4) as fA,
            ):
                xtu = xpool.tile([128, 2, SLAB_PAD], BF16, tag="xtu")
                nc.sync.dma_start(xtu[:], inp["xtu"][:])
                xti = xpool.tile([128, 2, SLAB_PAD], BF16, tag="xti")
                nc.sync.dma_start(xti[:], inp["xti"][:])
                for h, lo, hi in ((0, 0, H0W), (1, H0W, WPC)):
                    for r in RELS:
                        xs = xtu if SRC_IS_USER[r] else xti
                        for nt in range(lo, hi):
                            ps = psA.tile([128, D_HID], F32, tag="psA")
                            for kc in range(2):
                                nc.tensor.matmul(
                                    ps[:],
                                    xs[:, kc, nt * 128 : (nt + 1) * 128],
                                    w1_sb[r][:, kc, :],
                                    start=(kc == 0),
                                    stop=(kc == 1),
                                )
                            ft = fA.tile([128, D_HID], BF16, tag="fA")
                            nc.scalar.activation(
                                ft[:], ps[:], AF.Copy,
                                scale=rso_sb[r][:, nt : nt + 1],
                            )
                            nc.sync.dma_start(
                                f1[r][h][(nt - lo) * 128 : (nt - lo + 1) * 128, :],
                                ft[:],
                            )
                        ag(f1[r][h], f1f[r][h])

            # ---------------- phases B/C: SpMM passes
            ntmax = {
                h: max(meta[r]["ntmax"][h] for r in RELS) for h in (0, 1)
            }

            def _build_s(spool, tag, dst_ap, nt, ntm):
                """One DVE op: S[e, t, m] = (iota[m] == dstrel[e, t])."""
                st = spool.tile([128, ntm, 128], BF16, tag=tag, name=tag)
                nc.vector.tensor_tensor(
                    st[:, :nt, :],
                    iota_sb[:, :]
                    .rearrange("p (o f) -> p o f", o=1)
                    .broadcast_to([128, nt, 128]),
                    dst_ap.rearrange("p (t o) -> p t o", o=1)
                    .broadcast_to([128, nt, 128]),
                    mybir.AluOpType.is_equal,
                )
                return st

            def seg_pass(layer, gsrc, d, wg, gpool, pspool, spool, flush,
                         hook=None):
                """One gather+segment pass over all windows.

                gsrc[r][h]: DRAM table AP; d: feature dim; flush(w, psums)
                consumes the 3 accumulated PSUM tiles; hook(w) runs after
                each window's flush (used to emit mid-pass collectives)."""
                groups = _groups(wg)
                # fixed tag shapes: max tiles per group per (r, h)
                gmax = {
                    (r, h): max(
                        sum(meta[r]["nt"][h][w] for w in wl) for wl in groups
                    )
                    for r in RELS for h in (0, 1)
                }
                for wl in groups:
                    nw = len(wl)
                    w0 = wl[0]
                    gt = {}
                    for h in (0, 1):
                        for r in RELS:
                            m = meta[r]
                            ntl = sum(m["nt"][h][w] for w in wl)  # tiles
                            nsl = ntl * 128                       # slots
                            o0 = int(m["off"][h][w0])             # slot offset
                            ix = gpool.tile(
                                [128, gmax[(r, h)] * 8], I16,
                                tag=f"ix{layer}{h}",
                            )
                            nc.sync.dma_start(
                                ix[:, : nsl // 16],
                                inp[f"ix{h}_{r}"][:, o0 // 16 : (o0 + nsl) // 16],
                            )
                            g = gpool.tile(
                                [128, gmax[(r, h)], d], BF16,
                                tag=f"g{layer}{h}",
                            )
                            k0 = 0
                            while k0 < nsl:
                                k1 = min(k0 + MAXCALL, nsl)
                                nc.gpsimd.dma_gather(
                                    g[:, k0 // 128 : k1 // 128, :],
                                    gsrc[r][h][0 : TROWS[h], :],
                                    ix[:, k0 // 16 : k1 // 16],
                                    k1 - k0,
                                    k1 - k0,
                                    d,
                                    single_packet=True,
                                    queue_num=qctr[0] % 4,
                                )
                                qctr[0] += 1
                                k0 = k1
                            gt[(r, h)] = g
                    for j, w in enumerate(wl):
                        psums = {}
                        for r in RELS:
                            m = meta[r]
                            ps = pspool.tile(
                                [128, d], F32, tag=f"ps{layer}{r}",
                                name=f"ps{layer}{r}",
                            )
                            psums[r] = ps
                            first = True
                            for h in (0, 1):
                                ntw = int(m["nt"][h][w])
                                dso = int(m["off"][h][w]) // 128
                                tb = dso - int(m["off"][h][w0]) // 128
                                s = _build_s(
                                    spool, f"S{layer}{h}",
                                    ds_sb[r][h][:, dso : dso + ntw], ntw,
                                    ntmax[h],
                                )
                                g = gt[(r, h)]
                                for t in range(ntw):
                                    nc.tensor.matmul(
                                        ps[:],
                                        s[:, t, :],
                                        g[:, tb + t, :],
                                        start=first,
                                        stop=(h == 1 and t == ntw - 1),
                                    )
                                    first = False
                        flush(w, psums)
                        if hook is not None:
                            hook(w)

            with (
                tc.tile_pool(name="g1", bufs=4) as gpool1,
                tc.tile_pool(name="s1", bufs=3) as spool1,
                tc.tile_pool(name="fl", bufs=2) as flpool,
                tc.tile_pool(name="g2", bufs=3) as gpool2,
                tc.tile_pool(name="s2", bufs=3) as spool2,
            ):

                def flush1(w, psums):
                    t1 = flpool.tile([128, D_HID], F32, tag="t1")
                    nc.vector.scalar_tensor_tensor(
                        t1[:], psums["follows"][:], rsif[:, w : w + 1], b1u[:],
                        mult, add,
                    )
                    t2 = flpool.tile([128, D_HID], F32, tag="t2")
                    nc.vector.scalar_tensor_tensor(
                        t2[:], psums["ratedby"][:], rsirb[:, w : w + 1], t1[:],
                        mult, add,
                    )
                    hu = flpool.tile([128, D_HID], BF16, tag="hu")
                    nc.scalar.activation(hu[:], t2[:], AF.Relu)
                    t3 = flpool.tile([128, D_HID], F32, tag="t3")
                    nc.vector.scalar_tensor_tensor(
                        t3[:], psums["rates"][:], rsii[:, w : w + 1], b1i[:],
                        mult, add,
                    )
                    hi = flpool.tile([128, D_HID], BF16, tag="hi")
                    nc.scalar.activation(hi[:], t3[:], AF.Relu)
                    # transpose h tiles (PE) for the layer-2 feature matmuls
                    hts = {}
                    for nm, hh in (("u", hu), ("i", hi)):
                        ht = flpool.tile([128, 2, 128], BF16, tag=f"ht{nm}", name=f"ht{nm}")
                        for half in range(2):
                            pst = pstpool.tile([128, 128], BF16, tag="pst")
                            nc.tensor.transpose(
                                pst[:],
                                hh[:, half * 128 : (half + 1) * 128],
                                ident_sb[:],
                            )
                            nc.scalar.activation(ht[:, half, :], pst[:], AF.Copy)
                        hts[nm] = ht
                    hidx = 0 if w < H0W else 1
                    wl = w if w < H0W else w - H0W
                    for r in RELS:
                        ht = hts["u"] if SRC_IS_USER[r] else hts["i"]
                        ps2 = ps2pool.tile([128, D_OUT], F32, tag="ps2")
                        for kc in range(2):
                            nc.tensor.matmul(
                                ps2[:],
                                ht[:, kc, :],
                                w2_sb[r][:, kc, :],
                                start=(kc == 0),
                                stop=(kc == 1),
                            )
                        f2t = flpool.tile([128, D_OUT], BF16, tag="f2t")
                        nc.scalar.activation(
                            f2t[:], ps2[:], AF.Copy,
                            scale=rso_sb[r][:, w : w + 1],
                        )
                        nc.sync.dma_start(
                            f2[r][hidx][wl * 128 : (wl + 1) * 128, :], f2t[:]
                        )

                def hook1(w):
                    if w == H0W - 1:
                        for r in RELS:
                            ag(f2[r][0], f2f[r][0])
                    elif w == WPC - 1:
                        for r in RELS:
                            ag(f2[r][1], f2f[r][1])

                with (
                    tc.tile_pool(name="ps1", bufs=1, space="PSUM") as pspool1,
                    tc.tile_pool(name="pst", bufs=2, space="PSUM") as pstpool,
                    tc.tile_pool(name="ps2p", bufs=2, space="PSUM") as ps2pool,
                ):
                    seg_pass(1, f1f, D_HID, WG1, gpool1, pspool1, spool1,
                             flush1, hook1)

                def flush2(w, psums):
                    t1 = flpool.tile([128, D_OUT], F32, tag="o1")
                    nc.vector.scalar_tensor_tensor(
                        t1[:], psums["follows"][:], rsif[:, w : w + 1], b2u[:],
                        mult, add,
                    )
                    out_u = flpool.tile([128, D_OUT], F32, tag="ou")
                    nc.vector.scalar_tensor_tensor(
                        out_u[:], psums["ratedby"][:], rsirb[:, w : w + 1],
                        t1[:], mult, add,
                    )
                    nc.sync.dma_start(ou[w * 128 : (w + 1) * 128, :], out_u[:])
                    out_i = flpool.tile([128, D_OUT], F32, tag="oiT")
                    nc.vector.scalar_tensor_tensor(
                        out_i[:], psums["rates"][:], rsii[:, w : w + 1], b2i[:],
                        mult, add,
                    )
                    nc.sync.dma_start(oi[w * 128 : (w + 1) * 128, :], out_i[:])

                with tc.tile_pool(name="ps2c", bufs=2, space="PSUM") as pspool2:
                    seg_pass(2, f2f, D_OUT, WG2, gpool2, pspool2, spool2,
                             flush2)

    nc.compile()
    return nc


# ------------------------------------------------------------------- kernel


def prepare(inputs):
    """Host-side prep: returns (meta, in_maps)."""
    meta = {}
    for r in RELS:
        meta[r] = _prep_relation(inputs[f"src_{r}"], inputs[f"dst_{r}"])

    bf = ml_dtypes.bfloat16
    x_user = np.asarray(inputs["x_user"], np.float32)
    x_item = np.asarray(inputs["x_item"], np.float32)

    xtu = _xt_slabs(x_user, D_IN)
    xti = _xt_slabs(x_item, D_IN)

    rs_out = {}
    rs_in = {}
    for r in RELS:
        src = np.asarray(inputs[f"src_{r}"]).astype(np.int64)
        dst = np.asarray(inputs[f"dst_{r}"]).astype(np.int64)
        rs_out[r] = _rs_slabs(
            1.0
            / np.sqrt(np.maximum(np.bincount(src, minlength=N), 1.0)).astype(
                np.float32
            )
        )
        rs_in[r] = 1.0 / np.sqrt(
            np.maximum(np.bincount(dst, minlength=N), 1.0)
        ).astype(np.float32)

    rsif = _rs_slabs(0.5 * rs_in["follows"])
    rsirb = _rs_slabs(0.5 * rs_in["ratedby"])
    rsii = _rs_slabs(rs_in["rates"])

    iota = np.broadcast_to(np.arange(128, dtype=np.float32), (128, 128)).astype(bf)
    ident = np.eye(128, dtype=np.float32).astype(bf)

    b1u = np.broadcast_to(
        0.5
        * (
            np.asarray(inputs["b1_follows"], np.float32)
            + np.asarray(inputs["b1_ratedby"], np.float32)
        ),
        (128, D_HID),
    ).astype(np.float32)
    b1i = np.broadcast_to(
        np.asarray(inputs["b1_rates"], np.float32), (128, D_HID)
    ).astype(np.float32)
    b2u = np.broadcast_to(
        0.5
        * (
            np.asarray(inputs["b2_follows"], np.float32)
            + np.asarray(inputs["b2_ratedby"], np.float32)
        ),
        (128, D_OUT),
    ).astype(np.float32)
    b2i = np.broadcast_to(
        np.asarray(inputs["b2_rates"], np.float32), (128, D_OUT)
    ).astype(np.float32)

    w1 = {
        r: np.ascontiguousarray(
            np.asarray(inputs[f"W1_{r}"], np.float32)
            .astype(bf)
            .reshape(2, 128, D_HID)
            .transpose(1, 0, 2)
        )
        for r in RELS
    }
    w2 = {
        r: np.ascontiguousarray(
            np.asarray(inputs[f"W2_{r}"], np.float32)
            .astype(bf)
            .reshape(2, 128, D_OUT)
            .transpose(1, 0, 2)
        )
        for r in RELS
    }

    in_maps = []
    for c in range(NCORE):
        m = {
            "xtu": np.ascontiguousarray(xtu[c]),
            "xti": np.ascontiguousarray(xti[c]),
            "rsif": rsif[c],
            "rsirb": rsirb[c],
            "rsii": rsii[c],
            "iota": iota,
            "ident": ident,
            "b1u": b1u,
            "b1i": b1i,
            "b2u": b2u,
            "b2i": b2i,
        }
        for r in RELS:
            m[f"w1_{r}"] = w1[r]
            m[f"w2_{r}"] = w2[r]
            m[f"rso_{r}"] = rs_out[r][c]
            for h in (0, 1):
                m[f"ix{h}_{r}"] = meta[r]["idx"][h][c]
                m[f"ds{h}_{r}"] = meta[r]["dst"][h][c]
        in_maps.append(m)
    return meta, in_maps


def kernel(**inputs):
    key = tuple(
        (int(np.asarray(inputs[f"src_{r}"][:97]).sum()),
         int(np.asarray(inputs[f"dst_{r}"][:97]).sum()))
        for r in RELS
    )
    meta, in_maps = prepare(inputs)
    if key not in _CACHE:
        _CACHE[key] = _build(meta)
    nc = _CACHE[key]

    global LAST_RESULT
    res = run_bass_kernel_spmd(nc, in_maps, list(range(NCORE)))
    LAST_RESULT = res

    o_u = np.concatenate(
        [res.results[c]["ou"][:SLAB] for c in range(NCORE)], axis=0
    )
    o_i = np.concatenate(
        [res.results[c]["oi"][:SLAB] for c in range(NCORE)], axis=0
    )
    return (o_u, o_i)
